# revision 1
# baseline (speedup 1.0000x reference)
"""Llama MHA (B=2, S=2048, D=2048, H=16, causal, RoPE) on 8 trn2 cores.

Sharding: data-parallel over batch (2 groups of 4 cores) x tensor-parallel
over heads (4 heads per core). Each core computes, for its (batch, 4 heads):
  qT/kT = w^T x^T  (features on partitions, seq on free dim)
  RoPE on qT/kT (weights column-permuted on host so even/odd feature pairs
  land de-interleaved: rows 0:64 = even, 64:128 = odd; dot products are
  permutation-invariant so scores match the reference exactly)
  scoresT[k,q] blocks -> exp (no max subtraction needed: |score*scale| <~ 6)
  -> causal mask on diagonal blocks -> PV matmuls + all-ones-matrix
  denominator matmuls (every PSUM row = key-sum, so the broadcast for the
  normalization divide is free)
  -> normalize -> out projection partial resT = wo^T attnT.
Host sums the 4 partials per batch and transposes back.

All matmuls in bf16 (fp32 PSUM accumulation); softmax/normalization in fp32.
"""

import numpy as np
import ml_dtypes

import concourse.bass as bass
import concourse.mybir as mybir
import concourse.tile as tile
from concourse import bacc
from concourse.bass_utils import run_bass_kernel_spmd

B, S, D, H = 2, 2048, 2048, 16
DH = D // H            # 128 head dim
HPC = 4                # heads per core
N_CORES = 8
FH = HPC * DH          # 512 features per core
P = 128
KT = D // P            # 16 k-tiles over D
SC = S // 512          # 4 seq chunks of 512
ST = S // P            # 16 seq blocks of 128
THETA = 10000.0
SCALE = 1.0 / np.sqrt(DH)

DT = mybir.dt.bfloat16
NPDT = ml_dtypes.bfloat16

_prog_cache = {}


def _build():
    if "nc" in _prog_cache:
        return _prog_cache["nc"]
    nc = bacc.Bacc(None, target_bir_lowering=False, debug=False)

    xT = nc.dram_tensor("xT", [D, S], DT, kind="ExternalInput")
    wq = nc.dram_tensor("wq", [D, FH], DT, kind="ExternalInput")
    wk = nc.dram_tensor("wk", [D, FH], DT, kind="ExternalInput")
    wv = nc.dram_tensor("wv", [D, FH], DT, kind="ExternalInput")
    wo = nc.dram_tensor("wo", [FH, D], DT, kind="ExternalInput")
    cc = nc.dram_tensor("cc", [P, S], mybir.dt.float32, kind="ExternalInput")
    ss = nc.dram_tensor("ss", [P, S], mybir.dt.float32, kind="ExternalInput")
    masks = nc.dram_tensor("masks", [P, 4, 512], DT, kind="ExternalInput")
    resT = nc.dram_tensor("resT", [D, S], mybir.dt.float32, kind="ExternalOutput")

    f32 = mybir.dt.float32

    with tile.TileContext(nc) as tc:
        with (
            tc.tile_pool(name="persist", bufs=1) as pp,
            tc.tile_pool(name="psA", bufs=4, space="PSUM") as psA,
            tc.tile_pool(name="psO", bufs=2, space="PSUM") as psO,
            tc.tile_pool(name="psD", bufs=2, space="PSUM") as psD,
        ):
            qT = pp.tile([P, HPC, S], DT)     # per head: rows=feat, free=seq
            kT = pp.tile([P, HPC, S], DT)
            vn = pp.tile([P, ST, FH], DT)     # v natural: [seq-block, feat]
            attnT = pp.tile([P, HPC, S], DT)  # normalized attention output^T
            cc_t = pp.tile([P, S], f32)
            ss_t = pp.tile([P, S], f32)
            mask_t = pp.tile([P, 4, 512], DT)
            ones_mat = pp.tile([P, P], DT)    # denominator stationary: the
                                              # [128,128] all-ones matrix makes
                                              # every PSUM row the key-sum, so
                                              # the broadcast is free

            nc.vector.memset(ones_mat, 1.0)
            wo_t = pp.tile([P, HPC, D], DT)

            # ---------------- Phase 1: projections + RoPE -----------------
            with (
                tc.tile_pool(name="wpool", bufs=1) as wp,
                tc.tile_pool(name="xpool", bufs=2) as xp,
                tc.tile_pool(name="ropetmp", bufs=4) as rp,
            ):
                wq_t = wp.tile([P, KT, FH], DT)
                wk_t = wp.tile([P, KT, FH], DT)
                wv_t = wp.tile([P, KT, FH], DT)
                # DMA issue order is the Sync-queue order: interleave the
                # first x chunk with wq so the first matmul chain starts as
                # early as possible; defer everything not needed immediately.
                xc0 = xp.tile([P, KT, 512], DT, tag="xc", name="xc0")
                # first pieces are half-size so the first matmul starts sooner
                for gs in (slice(0, 2), slice(2, 4), slice(4, 8),
                           slice(8, 12), slice(12, 16)):
                    nc.sync.dma_start(
                        out=wq_t[:, gs, :],
                        in_=wq.rearrange("(kt p) f -> p kt f", p=P)[:, gs, :])
                    nc.sync.dma_start(
                        out=xc0[:, gs, :],
                        in_=xT.rearrange("(kt p) s -> p kt s", p=P)[:, gs, 0:512])
                nc.sync.dma_start(out=cc_t[:, 0:512], in_=cc[:, 0:512])
                nc.sync.dma_start(out=ss_t[:, 0:512], in_=ss[:, 0:512])
                for g in range(4):
                    gs = slice(g * 4, (g + 1) * 4)
                    nc.sync.dma_start(
                        out=wk_t[:, gs, :],
                        in_=wk.rearrange("(kt p) f -> p kt f", p=P)[:, gs, :])
                nc.sync.dma_start(out=wv_t, in_=wv.rearrange("(kt p) f -> p kt f", p=P))
                nc.sync.dma_start(out=mask_t, in_=masks[:, :, :])
                nc.sync.dma_start(out=cc_t[:, 512:], in_=cc[:, 512:])
                nc.sync.dma_start(out=ss_t[:, 512:], in_=ss[:, 512:])

                for sc in range(SC):
                    if sc == 0:
                        xc = xc0
                    else:
                        xc = xp.tile([P, KT, 512], DT, tag="xc", name=f"xc{sc}")
                        for g in range(4):
                            gs = slice(g * 4, (g + 1) * 4)
                            nc.sync.dma_start(
                                out=xc[:, gs, :],
                                in_=xT.rearrange("(kt p) s -> p kt s", p=P)[
                                    :, gs, sc * 512:(sc + 1) * 512],
                            )
                    if sc == 1:
                        nc.sync.dma_start(
                            out=wo_t, in_=wo.rearrange("(ft p) d -> p ft d", p=P))
                    csl = slice(sc * 512, (sc + 1) * 512)
                    # q/k projections with RoPE fused into the PSUM drain.
                    # All q chains before k chains: the first four only need
                    # wq+xc, giving the wk DMA ~14us of PE work as cover.
                    for wt, dst in ((wq_t, qT), (wk_t, kT)):
                        for h in range(HPC):
                            fsl = slice(h * DH, (h + 1) * DH)
                            pq = psA.tile([P, 512], f32, tag="ps", name=f"pq{sc}{h}")
                            for k in range(KT):
                                nc.tensor.matmul(
                                    pq, wt[:, k, fsl], xc[:, k, :],
                                    start=(k == 0), stop=(k == KT - 1),
                                )
                            # RoPE: dst = pq*cc + swap(pq)*(+/-ss)
                            # ss_t rows 0:64 = +sin (feeds bottom), rows
                            # 64:128 = -sin (feeds top); swap is done by
                            # writing each product into the opposite half
                            # so every DVE op has aligned base partitions.
                            ta = rp.tile([P, 512], f32, tag="ta")
                            tb = rp.tile([P, 512], f32, tag="tb")
                            nc.vector.tensor_mul(ta, pq, cc_t[:, csl])
                            nc.vector.tensor_mul(
                                tb[0:64, :], pq[64:128, :], ss_t[64:128, csl])
                            nc.vector.tensor_mul(
                                tb[64:128, :], pq[0:64, :], ss_t[0:64, csl])
                            nc.vector.tensor_add(dst[:, h, csl], ta, tb)
                    # v projection straight into natural layout
                    for st4 in range(4):
                        sb = sc * 4 + st4
                        pv = psA.tile([P, FH], f32, tag="ps", name=f"pv{sc}{st4}")
                        for k in range(KT):
                            nc.tensor.matmul(
                                pv, xc[:, k, st4 * P:(st4 + 1) * P], wv_t[:, k, :],
                                start=(k == 0), stop=(k == KT - 1),
                            )
                        nc.vector.tensor_copy(vn[:, sb, :], pv)

            # ---------------- Phase 2: attention ------------------------
            with (
                tc.tile_pool(name="ppool", bufs=8) as ptp,
                tc.tile_pool(name="npool", bufs=4) as np_,
            ):
                for qc in range(SC):
                    qsl = slice(qc * 512, (qc + 1) * 512)
                    for h in range(HPC):
                        fsl = slice(h * DH, (h + 1) * DH)
                        po = psO.tile([P, 512], f32, tag="po", name=f"po{h}{qc}")
                        pd = psD.tile([P, 512], f32, tag="pd", name=f"pd{h}{qc}")
                        nkb = 4 * qc + 4
                        for kb in range(nkb):
                            ps = psA.tile([P, 512], f32, tag="ps",
                                          name=f"ps{h}{qc}{kb}")
                            nc.tensor.matmul(
                                ps, kT[:, h, kb * P:(kb + 1) * P], qT[:, h, qsl],
                                start=True, stop=True,
                            )
                            pt = ptp.tile([P, 512], DT, tag="pt")
                            nc.scalar.activation(
                                pt, ps, mybir.ActivationFunctionType.Exp,
                                scale=float(SCALE),
                            )
                            if kb >= 4 * qc:
                                nc.vector.tensor_mul(
                                    pt, pt, mask_t[:, kb - 4 * qc, :])
                            nc.tensor.matmul(
                                po, vn[:, kb, fsl], pt,
                                start=(kb == 0), stop=(kb == nkb - 1),
                            )
                            nc.tensor.matmul(
                                pd, ones_mat, pt,
                                start=(kb == 0), stop=(kb == nkb - 1),
                            )
                        bc = np_.tile([P, 512], f32, tag="bc")
                        nc.vector.reciprocal_approx_fast(out=bc, in_=pd)
                        nc.vector.tensor_mul(attnT[:, h, qsl], po, bc)

            # ---------------- Phase 3: output projection ----------------
            with (
                tc.tile_pool(name="rpool", bufs=4) as rop,
            ):
                for db in range(KT):
                    rt = rop.tile([P, S], f32, tag="rt")
                    last = db == KT - 1
                    for sc in range(SC):
                        csl = slice(sc * 512, (sc + 1) * 512)
                        pr = psA.tile([P, 512], f32, tag="ps", name=f"pr{sc}{db}")
                        for ft in range(HPC):
                            nc.tensor.matmul(
                                pr, wo_t[:, ft, db * P:(db + 1) * P],
                                attnT[:, ft, csl],
                                start=(ft == 0), stop=(ft == HPC - 1),
                            )
                        nc.vector.tensor_copy(rt[:, csl], pr)
                        if last:
                            nc.sync.dma_start(
                                out=resT[db * P:(db + 1) * P, csl],
                                in_=rt[:, csl])
                    if not last:
                        nc.sync.dma_start(
                            out=resT[db * P:(db + 1) * P, :], in_=rt)

    nc.finalize()
    _prog_cache["nc"] = nc
    return nc


def _host_inputs(x, w_q, w_k, w_v, w_o):
    """Build the 8 per-core input maps."""
    # RoPE de-interleave permutation per head: evens then odds
    i = np.arange(DH)
    perm_head = np.concatenate([i[0::2], i[1::2]])  # within-head column order

    t = np.arange(S, dtype=np.float64)
    inv_freq = 1.0 / (THETA ** (np.arange(0, DH, 2, dtype=np.float64) / DH))
    ang = np.outer(t, inv_freq)          # [S, 64]
    cosT = np.cos(ang).T.astype(np.float32)   # [64, S]
    sinT = np.sin(ang).T.astype(np.float32)
    cc = np.vstack([cosT, cosT])         # [128, S]
    ss = np.vstack([sinT, -sinT])        # +sin feeds bottom half, -sin top

    # diagonal causal masks: mask[j][k, q] = 1 if 128*j + k <= q
    kk = np.arange(P)[:, None]
    qq = np.arange(512)[None, :]
    masks = np.stack(
        [(P * j + kk <= qq) for j in range(4)], axis=1
    ).astype(NPDT)                        # [128, 4, 512]

    in_maps = []
    for core in range(N_CORES):
        b = core // 4
        h0 = (core % 4) * HPC
        cols = np.concatenate(
            [h * DH + perm_head for h in range(h0, h0 + HPC)])   # rope-permuted
        vcols = np.arange(h0 * DH, (h0 + HPC) * DH)              # natural
        in_maps.append({
            "xT": np.ascontiguousarray(x[b].T).astype(NPDT),
            "wq": np.ascontiguousarray(w_q[:, cols]).astype(NPDT),
            "wk": np.ascontiguousarray(w_k[:, cols]).astype(NPDT),
            "wv": np.ascontiguousarray(w_v[:, vcols]).astype(NPDT),
            "wo": np.ascontiguousarray(w_o[vcols, :]).astype(NPDT),
            "cc": cc,
            "ss": ss,
            "masks": masks,
        })
    return in_maps


def kernel(x, w_q, w_k, w_v, w_o, _trace=False, _results_out=None):
    x = np.asarray(x, dtype=np.float32)
    w_q = np.asarray(w_q, dtype=np.float32)
    w_k = np.asarray(w_k, dtype=np.float32)
    w_v = np.asarray(w_v, dtype=np.float32)
    w_o = np.asarray(w_o, dtype=np.float32)
    nc = _build()
    in_maps = _host_inputs(x, w_q, w_k, w_v, w_o)
    res = run_bass_kernel_spmd(
        nc, in_maps, core_ids=list(range(N_CORES)), trace=_trace)
    if _results_out is not None:
        _results_out.append(res)
    out = np.empty((B, S, D), np.float32)
    for b in range(B):
        acc = res.results[4 * b]["resT"].astype(np.float32)
        for g in range(1, 4):
            acc = acc + res.results[4 * b + g]["resT"]
        out[b] = acc.T
    return out



# revision 14
# speedup vs baseline: 1.0943x; 1.0943x over previous
"""Llama MHA (B=2, S=2048, D=2048, H=16, causal, RoPE) on 8 trn2 cores.

Sharding: data-parallel over batch (2 groups of 4 cores) x tensor-parallel
over heads (4 heads per core). Single-core program per core; host splits
inputs and sums the 4 out-projection partials per batch.

v2 design notes (vs the phase-sequential baseline):
- Phases interleaved per 512-seq chunk: proj(sc) -> attn(sc) -> outproj(sc),
  with proj(sc+1)/outproj(sc-1) matmuls emitted as fillers inside the
  attention loop so the PE never starves while exp runs.
- exp is done on PAIRS of score blocks ([128,1024] across 2 PSUM banks),
  halving the scalar engine's fixed per-instruction overhead.
- softmax denominator: DVE bf16 accumulation of exp tiles + one ones-matmul
  per (head, chunk) -- replaces 160 PE ones-matmuls with 16.
- causal trim: PV matmuls / exp / acc adds only touch the un-masked column
  range of diagonal blocks.
- all PSUM->SBUF drains (q/k pre-RoPE, v, out-proj) run on GpSimd; scalar
  does only exp; RoPE runs on DVE in bf16 (2x mode).
- host pre-permutes every DRAM tensor into the exact SBUF tile layout so
  DMAs are contiguous; output is bf16.
"""

import numpy as np
import ml_dtypes

import concourse.bass as bass
import concourse.mybir as mybir
import concourse.tile as tile
from concourse import bacc
from concourse.bass_utils import run_bass_kernel_spmd

B, S, D, H = 2, 2048, 2048, 16
DH = D // H            # 128 head dim
HPC = 4                # heads per core
N_CORES = 8
FH = HPC * DH          # 512 features per core
P = 128
KT = D // P            # 16 k-tiles over D
SC = S // 512          # 4 seq chunks of 512
ST = S // P            # 16 seq blocks of 128
THETA = 10000.0
SCALE = 1.0 / np.sqrt(DH)

DT = mybir.dt.bfloat16
F32 = mybir.dt.float32
NPDT = ml_dtypes.bfloat16

_prog_cache = {}


class FillerQueue:
    """Deferred PE work (closures) popped between attention pairs."""

    def __init__(self):
        self.q = []

    def push(self, fn, cost=1):
        self.q.append((fn, cost))

    def pop_cost(self, budget):
        while budget > 0 and self.q:
            fn, cost = self.q.pop(0)
            fn()
            budget -= cost

    def drain(self):
        while self.q:
            fn, _ = self.q.pop(0)
            fn()


def _build():
    if "nc" in _prog_cache:
        return _prog_cache["nc"]
    nc = bacc.Bacc(None, target_bir_lowering=False, debug=False)

    xd = nc.dram_tensor("xd", [SC, P, KT, 512], DT, kind="ExternalInput")
    wqd = nc.dram_tensor("wqd", [HPC, P, KT, DH], DT, kind="ExternalInput")
    wkd = nc.dram_tensor("wkd", [HPC, P, KT, DH], DT, kind="ExternalInput")
    wvd = nc.dram_tensor("wvd", [P, KT, FH], DT, kind="ExternalInput")
    wod = nc.dram_tensor("wod", [P, HPC, D], DT, kind="ExternalInput")
    ccd = nc.dram_tensor("ccd", [P, S], DT, kind="ExternalInput")
    ssd = nc.dram_tensor("ssd", [P, S], DT, kind="ExternalInput")
    trid = nc.dram_tensor("trid", [P, P], DT, kind="ExternalInput")
    resT = nc.dram_tensor("resT", [KT, SC, P, 512], DT, kind="ExternalOutput")

    with tile.TileContext(nc) as tc:
        with (
            tc.tile_pool(name="persist", bufs=1) as pp,
            tc.tile_pool(name="qtc", bufs=2) as qp,
            tc.tile_pool(name="attnc", bufs=2) as ap,
            tc.tile_pool(name="pqb", bufs=8) as bp,
            tc.tile_pool(name="rope", bufs=2) as rp,
            tc.tile_pool(name="pt", bufs=3) as tp,
            tc.tile_pool(name="accp", bufs=2) as cp,
            tc.tile_pool(name="bcp", bufs=2) as vp,
            tc.tile_pool(name="rtp", bufs=3) as op_,
            tc.tile_pool(name="xcp", bufs=2) as xp,
            tc.tile_pool(name="psP", bufs=2, space="PSUM") as psP,
            tc.tile_pool(name="psA", bufs=2, space="PSUM") as psA,
            tc.tile_pool(name="psB", bufs=2, space="PSUM") as psB,
        ):
            kT = pp.tile([P, HPC, S], DT)      # rope'd k^T, all chunks
            vn = pp.tile([P, ST, FH], DT)      # v natural [seq-block, feat]
            cc_t = pp.tile([P, S], DT)
            ss_t = pp.tile([P, S], DT)
            tri = pp.tile([P, P], DT)          # tri[k,u] = (k <= u)
            ones_mat = pp.tile([P, P], DT)
            wq_t = pp.tile([P, HPC, KT, DH], DT)
            wk_t = pp.tile([P, HPC, KT, DH], DT)
            wv_t = pp.tile([P, KT, FH], DT)
            wo_t = pp.tile([P, HPC, D], DT)

            nc.vector.memset(ones_mat, 1.0)

            # ---------------- preamble DMAs --------------------------------
            # One strictly-ordered queue (scalar HWDGE -- avoids the sync
            # engine's init backlog) so critical pieces transfer first at
            # full bandwidth. Outputs go on the sync queue.
            xc = [None] * SC
            xc[0] = xp.tile([P, KT, 512], DT, tag="xc", name="xc0")
            nc.scalar.dma_start(out=wq_t[:, 0, 0:4, :], in_=wqd[0, :, 0:4, :])
            nc.scalar.dma_start(out=xc[0][:, 0:2, :], in_=xd[0, :, 0:2, :])
            nc.scalar.dma_start(out=xc[0][:, 2:4, :], in_=xd[0, :, 2:4, :])
            nc.scalar.dma_start(out=wq_t[:, 0, 4:16, :], in_=wqd[0, :, 4:16, :])
            nc.scalar.dma_start(out=xc[0][:, 4:8, :], in_=xd[0, :, 4:8, :])
            nc.scalar.dma_start(out=xc[0][:, 8:16, :], in_=xd[0, :, 8:16, :])
            for h in range(1, HPC):
                nc.scalar.dma_start(out=wq_t[:, h, :, :], in_=wqd[h, :, :, :])
            nc.scalar.dma_start(out=cc_t[:, 0:512], in_=ccd[:, 0:512])
            nc.scalar.dma_start(out=ss_t[:, 0:512], in_=ssd[:, 0:512])
            for h in range(HPC):
                nc.scalar.dma_start(out=wk_t[:, h, :, :], in_=wkd[h, :, :, :])
            nc.scalar.dma_start(out=wv_t, in_=wvd[:, :, :])
            nc.scalar.dma_start(out=cc_t[:, 512:], in_=ccd[:, 512:])
            nc.scalar.dma_start(out=ss_t[:, 512:], in_=ssd[:, 512:])
            nc.scalar.dma_start(out=tri, in_=trid[:, :])
            xc[1] = xp.tile([P, KT, 512], DT, tag="xc", name="xc1")
            nc.scalar.dma_start(out=xc[1], in_=xd[1, :, :, :])
            nc.scalar.dma_start(out=wo_t, in_=wod[:, :, :])

            qTc = [None] * SC    # current-chunk rope'd q
            attnc = [None] * SC  # current-chunk attention output

            def emit_chain_matmul(ps, wt, h, k, xcc):
                nc.tensor.matmul(
                    ps, wt[:, h, k, :], xcc[:, k, :],
                    start=(k == 0), stop=(k == KT - 1),
                )

            def emit_vchain_matmul(ps, st4, k, xcc):
                nc.tensor.matmul(
                    ps, xcc[:, k, st4 * P:(st4 + 1) * P], wv_t[:, k, :],
                    start=(k == 0), stop=(k == KT - 1),
                )

            def make_proj_closures(sc, fq):
                """Queue proj(sc)'s chain matmuls + drains as fillers."""
                csl = slice(sc * 512, (sc + 1) * 512)
                state = {}

                def start_chain(key):
                    ps = psP.tile([P, 512], F32, tag="ps", name=f"ps_{key}_{sc}")
                    state[key] = ps
                    return ps

                for wt, kind in ((wq_t, "q"), (wk_t, "k")):
                    for h in range(HPC):
                        key = f"{kind}{h}"
                        for k in range(KT):
                            def mm(k=k, h=h, wt=wt, key=key):
                                ps = state[key] if k else start_chain(key)
                                emit_chain_matmul(ps, wt, h, k, xc[sc])
                            fq.push(mm, 1)

                        def drain(key=key, kind=kind, h=h):
                            pqb = bp.tile([P, 512], DT, tag="pqb",
                                          name=f"pqb_{key}_{sc}")
                            nc.scalar.activation(
                                pqb, state[key],
                                mybir.ActivationFunctionType.Copy)
                            state[key + "_b"] = pqb
                        fq.push(drain, 0)
                for st4 in range(4):
                    key = f"v{st4}"
                    for k in range(KT):
                        def mm(k=k, st4=st4, key=key):
                            ps = state[key] if k else start_chain(key)
                            emit_vchain_matmul(ps, st4, k, xc[sc])
                        fq.push(mm, 1)

                    def drain(st4=st4, key=key):
                        nc.scalar.activation(
                            vn[:, sc * 4 + st4, :], state[key],
                            mybir.ActivationFunctionType.Copy)
                    fq.push(drain, 0)

                def emit_rope():
                    qTc[sc] = qp.tile([P, HPC, 512], DT, tag="qt",
                                      name=f"qt{sc}")
                    for kind in ("q", "k"):
                        for h in range(HPC):
                            pqb = state[f"{kind}{h}_b"]
                            ta = rp.tile([P, 512], DT, tag="ta")
                            tb = rp.tile([P, 512], DT, tag="tb")
                            nc.vector.tensor_mul(ta, pqb, cc_t[:, csl])
                            nc.vector.tensor_mul(
                                tb[0:64, :], pqb[64:128, :], ss_t[64:128, csl])
                            nc.vector.tensor_mul(
                                tb[64:128, :], pqb[0:64, :], ss_t[0:64, csl])
                            dst = qTc[sc][:, h, :] if kind == "q" else kT[:, h, csl]
                            nc.vector.tensor_add(dst, ta, tb)

                return emit_rope

            def emit_attn(sc, fq):
                """Attention for q-chunk sc; pops fillers between pairs."""
                nkb = 4 * (sc + 1)
                attnc[sc] = ap.tile([P, HPC, 512], DT, tag="at", name=f"at{sc}")
                for h in range(HPC):
                    fsl = slice(h * DH, (h + 1) * DH)
                    po = psA.tile([P, 512], F32, tag="po", name=f"po{sc}{h}")
                    acc = cp.tile([P, 512], DT, tag="acc", name=f"acc{sc}{h}")
                    for pi in range(nkb // 2):
                        kb0 = 2 * pi
                        # j-index of each block on the block-diagonal (<0: off)
                        j0 = kb0 - 4 * sc
                        j1 = kb0 + 1 - 4 * sc
                        o0 = max(0, 128 * j0)
                        o1 = max(0, 128 * j1)
                        pb = psB.tile([P, 1024], F32, tag="pb",
                                      name=f"pb{sc}{h}{pi}")
                        nc.tensor.matmul(
                            pb[:, o0:512], kT[:, h, kb0 * P:(kb0 + 1) * P],
                            qTc[sc][:, h, o0:512], start=True, stop=True,
                        )
                        nc.tensor.matmul(
                            pb[:, 512:1024], kT[:, h, (kb0 + 1) * P:(kb0 + 2) * P],
                            qTc[sc][:, h, :], start=True, stop=True,
                        )
                        fq.pop_cost(2)
                        pt2 = tp.tile([P, 1024], DT, tag="pt",
                                      name=f"pt{sc}{h}{pi}")
                        nc.scalar.activation(
                            pt2[:, o0:], pb[:, o0:],
                            mybir.ActivationFunctionType.Exp, scale=float(SCALE),
                        )
                        # mask diagonal blocks (within-block triangle)
                        if j0 >= 0:
                            nc.vector.tensor_mul(
                                pt2[:, o0:o0 + 128], pt2[:, o0:o0 + 128], tri)
                        if j1 >= 0:
                            nc.vector.tensor_mul(
                                pt2[:, 512 + o1:512 + o1 + 128],
                                pt2[:, 512 + o1:512 + o1 + 128], tri)
                        # PV matmuls + denominator accumulation
                        for kb, off, o in ((kb0, 0, o0), (kb0 + 1, 512, o1)):
                            nc.tensor.matmul(
                                po[:, o:512], vn[:, kb, fsl],
                                pt2[:, off + o:off + 512],
                                start=(kb == 0), stop=(kb == nkb - 1),
                                skip_group_check=True,
                            )
                            if kb == 0:
                                nc.vector.tensor_copy(acc, pt2[:, 0:512])
                            else:
                                nc.vector.tensor_add(
                                    acc[:, o:], acc[:, o:],
                                    pt2[:, off + o:off + 512])
                    pdw = psB.tile([P, 1024], F32, tag="pb", name=f"pd{sc}{h}")
                    pd = pdw[:, 0:512]
                    nc.tensor.matmul(pd, ones_mat, acc, start=True, stop=True)
                    bc = vp.tile([P, 512], F32, tag="bc", name=f"bc{sc}{h}")
                    nc.vector.reciprocal_approx_fast(out=bc, in_=pd)
                    nc.vector.tensor_mul(attnc[sc][:, h, :], po, bc)

            def make_outproj_closures(sc, fq, alt_pool=False):
                """Queue outproj(sc): per db: 4 matmuls + drain + DMA."""
                state = {}
                for db in range(KT):
                    key = f"o{db}"
                    for ft in range(HPC):
                        def mm(ft=ft, db=db, key=key):
                            if ft == 0:
                                pool = psA if (alt_pool and db % 2) else psP
                                tg = "po" if (alt_pool and db % 2) else "ps"
                                state[key] = pool.tile(
                                    [P, 512], F32, tag=tg,
                                    name=f"pr{sc}{db}")
                            nc.tensor.matmul(
                                state[key],
                                wo_t[:, ft, db * P:(db + 1) * P],
                                attnc[sc][:, ft, :],
                                start=(ft == 0), stop=(ft == HPC - 1),
                            )
                        fq.push(mm, 1)

                    def drain(db=db, key=key, sc=sc):
                        rt = op_.tile([P, 512], DT, tag="rt",
                                      name=f"rt{sc}{db}")
                        if db % 2 == 0:
                            nc.scalar.activation(
                                rt, state[key],
                                mybir.ActivationFunctionType.Copy)
                        else:
                            nc.vector.tensor_copy(rt, state[key])
                        nc.sync.dma_start(out=resT[db, sc, :, :], in_=rt)
                    fq.push(drain, 0)

            # ---------------- main interleaved program --------------------
            fq = FillerQueue()

            # proj(0) solid
            mk_rope0 = make_proj_closures(0, fq)
            fq.drain()
            mk_rope0()

            for sc in range(SC):
                # stage next chunk's proj + previous chunk's outproj
                if sc + 1 < SC:
                    if sc + 2 < SC:
                        xc[sc + 2] = xp.tile([P, KT, 512], DT, tag="xc",
                                             name=f"xc{sc + 2}")
                        nc.scalar.dma_start(out=xc[sc + 2],
                                            in_=xd[sc + 2, :, :, :])
                    mk_rope = make_proj_closures(sc + 1, fq)
                if sc >= 1:
                    make_outproj_closures(sc - 1, fq)

                emit_attn(sc, fq)

                fq.drain()
                if sc + 1 < SC:
                    mk_rope()

            make_outproj_closures(SC - 1, fq, alt_pool=True)
            fq.drain()

    nc.finalize()
    _prog_cache["nc"] = nc
    return nc


def _host_inputs(x, w_q, w_k, w_v, w_o):
    """Build the 8 per-core input maps (DRAM laid out as SBUF tile images)."""
    i = np.arange(DH)
    perm_head = np.concatenate([i[0::2], i[1::2]])  # de-interleave pairs

    t = np.arange(S, dtype=np.float64)
    inv_freq = 1.0 / (THETA ** (np.arange(0, DH, 2, dtype=np.float64) / DH))
    ang = np.outer(t, inv_freq)               # [S, 64]
    cosT = np.cos(ang).T
    sinT = np.sin(ang).T
    ccd = np.vstack([cosT, cosT]).astype(NPDT)    # [128, S]
    ssd = np.vstack([sinT, -sinT]).astype(NPDT)   # +sin bottom, -sin top

    kk = np.arange(P)[:, None]
    uu = np.arange(P)[None, :]
    trid = (kk <= uu).astype(NPDT)            # [128, 128]

    in_maps = []
    for core in range(N_CORES):
        b = core // 4
        h0 = (core % 4) * HPC
        cols = np.concatenate(
            [(h0 + h) * DH + perm_head for h in range(HPC)])
        vcols = np.arange(h0 * DH, (h0 + HPC) * DH)

        wq_c = w_q[:, cols].reshape(KT, P, HPC, DH).transpose(2, 1, 0, 3)
        wk_c = w_k[:, cols].reshape(KT, P, HPC, DH).transpose(2, 1, 0, 3)
        wv_c = w_v[:, vcols].reshape(KT, P, FH).transpose(1, 0, 2)
        wo_c = w_o[vcols, :].reshape(HPC, P, D).transpose(1, 0, 2)
        x_c = x[b].reshape(SC, 512, KT, P).transpose(0, 3, 2, 1)

        in_maps.append({
            "xd": np.ascontiguousarray(x_c).astype(NPDT),
            "wqd": np.ascontiguousarray(wq_c).astype(NPDT),
            "wkd": np.ascontiguousarray(wk_c).astype(NPDT),
            "wvd": np.ascontiguousarray(wv_c).astype(NPDT),
            "wod": np.ascontiguousarray(wo_c).astype(NPDT),
            "ccd": ccd,
            "ssd": ssd,
            "trid": trid,
        })
    return in_maps


def kernel(x, w_q, w_k, w_v, w_o, _trace=False, _results_out=None):
    x = np.asarray(x, dtype=np.float32)
    w_q = np.asarray(w_q, dtype=np.float32)
    w_k = np.asarray(w_k, dtype=np.float32)
    w_v = np.asarray(w_v, dtype=np.float32)
    w_o = np.asarray(w_o, dtype=np.float32)
    nc = _build()
    in_maps = _host_inputs(x, w_q, w_k, w_v, w_o)
    res = run_bass_kernel_spmd(
        nc, in_maps, core_ids=list(range(N_CORES)), trace=_trace)
    if _results_out is not None:
        _results_out.append(res)
    out = np.empty((B, S, D), np.float32)
    for b in range(B):
        acc = res.results[4 * b]["resT"].astype(np.float32)
        for g in range(1, 4):
            acc = acc + res.results[4 * b + g]["resT"].astype(np.float32)
        # resT [KT, SC, P, 512] -> [S, D]
        out[b] = acc.transpose(1, 3, 0, 2).reshape(S, D)
    return out
